# revision 3
# baseline (speedup 1.0000x reference)
"""BCC-lattice grid encoding (embedding lookup) on 8 Trainium2 NeuronCores.

Strategy: points batch-sharded across 8 cores, 512MB grid table replicated.
Per core: a fp32 DVE pipeline computes the 4 BCC tetrahedron vertex row
indices + barycentric weights bit-exactly vs the jax reference (verified:
indices match the reference exactly, final output max abs err ~9e-9 from
fp reassociation only), rows are gathered from HBM with indirect DMAs
(128 rows per instruction — the only int32-index gather primitive on this
HW; dma_gather needs int16 indices + 256B strides), and the weighted
combination runs on the vector engine.

Measured on trn2: ~10.4 ms/core (8 cores run the same program in parallel),
dominated by the 8192 indirect-DMA instructions/core at ~1.12-1.3 us each of
Q7 SWDGE descriptor-generation time; the gathered bytes themselves are only
~34 MB/core (~0.1 ms at HBM rate). Tuning notes from iteration: gather-buffer
depth dbufs=16 with cg=32 chunks beats shallower buffering (15.2 -> 10.4 ms);
issuing the 4 vertices' gathers v-major (all of d1's chunk, then d2's...)
beats k-major interleaving by 2x; keeping the gather stream phase-separated
from the index pipeline (whole-core idx tensors) beats per-tile interleaving
by 3x (fine-grained cross-engine waits throttle the Pool queue); a larger
SWDGE descriptor-ring carveout and multi-queue round-robin change nothing.
"""
import os
import sys

for _p in ("/opt/trn_rl_repo", "/root/.axon_site/_ro/trn_rl_repo"):
    if os.path.isdir(_p) and _p not in sys.path:
        sys.path.insert(0, _p)

import numpy as np
import concourse.bass as bass
import concourse.bacc as bacc
import concourse.mybir as mybir
import concourse.tile as tile
from concourse.bass_utils import run_bass_kernel_spmd

f32 = mybir.dt.float32
i32 = mybir.dt.int32
A = mybir.AluOpType

N = 2_097_152          # total points
NCORES = 8
NSH = N // NCORES      # points per core
P = 128                # SBUF partitions
R3 = 16_777_216        # grid rows (256^3)
D = 8                  # channels per row
MAGIC = 8388608.0      # 2^23: (x + MAGIC) - MAGIC == rne-round-to-int


def _build_nc(nsh=NSH, fc=256, cg=32, dbufs=16, scratch=16384):
    """Build the per-core SPMD program.

    nsh: points this core owns; fc: index-pipeline tile width (free dim per
    partition); cg: gather/interp chunk width.
    """
    T = nsh // P                      # free-dim points per partition
    assert T % fc == 0 and T % cg == 0

    nc = bacc.Bacc(dynamic_dma_scratch_size=scratch)
    pts_in = nc.declare_dram_parameter("pts", [nsh, 3], f32, isOutput=False)
    grid_in = nc.declare_dram_parameter("grid", [R3, D], f32, isOutput=False)
    out_dram = nc.declare_dram_parameter("out", [nsh, D], f32, isOutput=True)

    # DRAM views: partition p owns points [p*T, (p+1)*T)
    pts_v = pts_in[:].rearrange("(p t) c -> p (t c)", p=P)     # [128, T*3]
    out_v = out_dram[:].rearrange("(p t) c -> p (t c)", p=P)   # [128, T*8]

    with tile.TileContext(nc) as tc:
        with (
            tc.tile_pool(name="persist", bufs=1) as pp,
            tc.tile_pool(name="scratch", bufs=1) as sp,
            tc.tile_pool(name="io", bufs=2) as iop,
            tc.tile_pool(name="dp", bufs=dbufs) as dpool,
        ):

            def ts(out, in0, s1, op0, s2=None, op1=None):
                if s2 is None:
                    nc.vector.tensor_scalar(out=out, in0=in0, scalar1=s1,
                                            scalar2=None, op0=op0)
                else:
                    nc.vector.tensor_scalar(out=out, in0=in0, scalar1=s1,
                                            scalar2=s2, op0=op0, op1=op1)

            def tt(out, in0, in1, op):
                nc.vector.tensor_tensor(out=out, in0=in0, in1=in1, op=op)

            # whole-core persistent index/weight tensors (phase separation:
            # keeping the 8192-instruction gather stream free of fine-grained
            # cross-engine waits measures ~3x faster than interleaving)
            idx = [pp.tile([P, T], i32, name=f"idx{v}", tag=f"idx{v}")
                   for v in range(4)]
            wt = [pp.tile([P, T], f32, name=f"w{v}", tag=f"w{v}")
                  for v in range(4)]

            ntile = T // fc
            for j in range(ntile):
                sl = slice(j * fc, (j + 1) * fc)
                pts_t = iop.tile([P, fc * 3], f32, name="pts_t", tag="pts")
                nc.sync.dma_start(out=pts_t[:],
                                  in_=pts_v[:, j * fc * 3:(j + 1) * fc * 3])
                p3 = pts_t[:].rearrange("p (t c) -> p t c", c=3)

                def st(tag):
                    return sp.tile([P, fc], f32, name=tag, tag=tag)

                # stage A: coords -> abc -> floors/fracs -> t,u,w
                xs, ys, zs = st("xs"), st("ys"), st("zs")
                ts(xs[:], p3[:, :, 0], 255.5, A.mult)
                ts(ys[:], p3[:, :, 1], 255.5, A.mult)
                ts(zs[:], p3[:, :, 2], 127.5, A.mult)
                av, bv, cv = st("av"), st("bv"), st("cv")
                tt(av[:], xs[:], ys[:], A.add)
                tt(bv[:], xs[:], zs[:], A.add)
                tt(cv[:], ys[:], zs[:], A.add)

                # floor via magic rne + correction (values >= 0)
                def floor_to(dst, x, rr, gg):
                    ts(rr[:], x[:], MAGIC, A.add, MAGIC, A.subtract)
                    tt(gg[:], rr[:], x[:], A.is_gt)
                    tt(dst[:], rr[:], gg[:], A.subtract)

                fa, fb, fcr = st("fa"), st("fb"), st("fc")
                Fa, Fb, Fc = st("Fa"), st("Fb"), st("Fc")
                h1, h2 = st("h1"), st("h2")
                floor_to(Fa, av, h1, h2)
                tt(fa[:], av[:], Fa[:], A.subtract)
                floor_to(Fb, bv, h1, h2)
                tt(fb[:], bv[:], Fb[:], A.subtract)
                floor_to(Fc, cv, h1, h2)
                tt(fcr[:], cv[:], Fc[:], A.subtract)

                tv, uv, wv = st("tv"), st("uv"), st("wv")
                tt(h1[:], Fb[:], Fc[:], A.subtract)      # d = Fb-Fc
                tt(tv[:], Fa[:], h1[:], A.add)
                tt(uv[:], Fa[:], h1[:], A.subtract)
                tt(h2[:], Fb[:], Fc[:], A.add)           # s = Fb+Fc
                tt(wv[:], h2[:], Fa[:], A.subtract)

                # stage B: barycentric weights
                s1t, s3t, s2t = st("s1t"), st("s3t"), st("s2t")
                tt(s1t[:], fa[:], fb[:], A.max)
                tt(s1t[:], s1t[:], fcr[:], A.max)
                tt(s3t[:], fa[:], fb[:], A.min)
                tt(s3t[:], s3t[:], fcr[:], A.min)
                tt(s2t[:], fa[:], fb[:], A.add)
                tt(s2t[:], s2t[:], fcr[:], A.add)
                tt(s2t[:], s2t[:], s1t[:], A.subtract)
                tt(s2t[:], s2t[:], s3t[:], A.subtract)
                # out = w0*d1 + w1*d2 + w2*d3 + w3*d4
                ts(wt[0][:, sl], s1t[:], -1.0, A.mult, 1.0, A.add)   # 1-s1
                nc.vector.tensor_copy(out=wt[1][:, sl], in_=s3t[:])  # s3
                tt(wt[2][:, sl], s1t[:], s2t[:], A.subtract)         # s1-s2
                tt(wt[3][:, sl], s2t[:], s3t[:], A.subtract)         # s2-s3

                # stage C: argmax/argmin one-hots (first-index tie-break)
                e1a, e1b, qe = st("e1a"), st("e1b"), st("qe")
                tt(h1[:], fa[:], fb[:], A.is_ge)          # gab
                tt(h2[:], fa[:], fcr[:], A.is_ge)         # gac
                tt(e1a[:], h1[:], h2[:], A.mult)
                ts(h1[:], h1[:], -1.0, A.mult, 1.0, A.add)  # gba = 1-gab
                tt(h2[:], fb[:], fcr[:], A.is_ge)         # gbc
                tt(e1b[:], h1[:], h2[:], A.mult)
                tt(qe[:], e1a[:], e1b[:], A.add)          # e1c = 1-qe
                ma, mb, qm = st("ma"), st("mb"), st("qm")
                tt(h1[:], fa[:], fb[:], A.is_le)          # lab
                tt(h2[:], fa[:], fcr[:], A.is_le)         # lac
                tt(ma[:], h1[:], h2[:], A.mult)
                ts(h1[:], h1[:], -1.0, A.mult, 1.0, A.add)  # lba
                tt(h2[:], fb[:], fcr[:], A.is_le)         # lbc
                tt(mb[:], h1[:], h2[:], A.mult)
                tt(qm[:], ma[:], mb[:], A.add)            # mc = 1-qm

                # stage D helpers
                def cfh(dst, x, bias):
                    """dst = floor(clamp(x*0.5 + bias, 0, 255.5)) ; dst f32"""
                    if bias == 0.0:
                        ts(h1[:], x[:], 0.5, A.mult)
                    else:
                        ts(h1[:], x[:], 0.5, A.mult, bias, A.add)
                    ts(h1[:], h1[:], 255.5, A.min, 0.0, A.max)
                    ts(h2[:], h1[:], MAGIC, A.add, MAGIC, A.subtract)
                    tt(h3[:], h2[:], h1[:], A.is_gt)
                    tt(dst[:], h2[:], h3[:], A.subtract)

                h3, h4 = st("h3"), st("h4")
                i0c, i1c, i2c = st("i0c"), st("i1c"), st("i2c")

                def combine(v):
                    ts(h1[:], i0c[:], 65536.0, A.mult)
                    ts(h2[:], i1c[:], 256.0, A.mult)
                    tt(h1[:], h1[:], h2[:], A.add)
                    tt(h1[:], h1[:], i2c[:], A.add)
                    nc.vector.tensor_copy(out=idx[v][:, sl], in_=h1[:])

                # vertex 1: floors of (t/2, u/2), w
                cfh(i0c, tv, 0.0)
                cfh(i1c, uv, 0.0)
                ts(i2c[:], wv[:], 255.0, A.min, 0.0, A.max)
                combine(0)
                # vertex 2: +(1,1,1) -> ((t+1)/2, (u+1)/2, w+1)
                cfh(i0c, tv, 0.5)
                cfh(i1c, uv, 0.5)
                ts(i2c[:], wv[:], 1.0, A.add, 255.0, A.min)
                ts(i2c[:], i2c[:], 0.0, A.max)
                combine(1)
                # vertex 3: p1 + abc_to_xyz(e1):
                # dt=1-2*e1c=2*qe-1, du=1-2*e1b, dw=1-2*e1a
                ts(h4[:], qe[:], 2.0, A.mult, -1.0, A.add)
                tt(h4[:], tv[:], h4[:], A.add)
                cfh(i0c, h4, 0.0)
                ts(h4[:], e1b[:], -2.0, A.mult, 1.0, A.add)
                tt(h4[:], uv[:], h4[:], A.add)
                cfh(i1c, h4, 0.0)
                ts(h4[:], e1a[:], -2.0, A.mult, 1.0, A.add)
                tt(h4[:], wv[:], h4[:], A.add)
                ts(i2c[:], h4[:], 255.0, A.min, 0.0, A.max)
                combine(2)
                # vertex 4: +2*unit(argmin): dt=2*mc=2-2*qm, du=2*mb, dw=2*ma
                ts(h4[:], qm[:], -2.0, A.mult, 2.0, A.add)
                tt(h4[:], tv[:], h4[:], A.add)
                cfh(i0c, h4, 0.0)
                ts(h4[:], mb[:], 2.0, A.mult)
                tt(h4[:], uv[:], h4[:], A.add)
                cfh(i1c, h4, 0.0)
                ts(h4[:], ma[:], 2.0, A.mult)
                tt(h4[:], wv[:], h4[:], A.add)
                ts(i2c[:], h4[:], 255.0, A.min, 0.0, A.max)
                combine(3)

            # phase 2: gather + interpolate, chunk by chunk. One indirect
            # DMA per vertex per chunk: the [P, cg] offset AP generates
            # P*cg descriptors in a single instruction, amortizing the
            # ~1us SWDGE fixed overhead that dominated the per-column
            # version (128 descriptors/instruction).
            nchunk = T // cg
            for ci in range(nchunk):
                dts = [dpool.tile([P, cg * D], f32, name=f"d{v}",
                                  tag=f"d{v}") for v in range(4)]
                for v in range(4):
                    nc.gpsimd.indirect_dma_start(
                        out=dts[v][:],
                        out_offset=None,
                        in_=grid_in[:],
                        in_offset=bass.IndirectOffsetOnAxis(
                            ap=idx[v][:, ci * cg:(ci + 1) * cg], axis=0),
                    )
                oc = iop.tile([P, cg * D], f32, name="oc", tag="oc")
                t2 = iop.tile([P, cg * D], f32, name="t2i", tag="t2i")
                for v in range(4):
                    wb = wt[v][:, ci * cg:(ci + 1) * cg].unsqueeze(-1) \
                        .broadcast_to([P, cg, D])
                    dv3 = dts[v][:].rearrange("p (t c) -> p t c", c=D)
                    if v == 0:
                        tt(oc[:].rearrange("p (t c) -> p t c", c=D),
                           dv3, wb, A.mult)
                    else:
                        tt(t2[:].rearrange("p (t c) -> p t c", c=D),
                           dv3, wb, A.mult)
                        tt(oc[:], oc[:], t2[:], A.add)
                nc.sync.dma_start(out=out_v[:, ci * cg * D:(ci + 1) * cg * D],
                                  in_=oc[:])

    nc.compile()
    return nc


_NC_CACHE = {}


def _get_nc(key=(NSH, 256, 128, 4)):
    if key not in _NC_CACHE:
        _NC_CACHE[key] = _build_nc(*key)
    return _NC_CACHE[key]


def kernel(pts: np.ndarray, grid: np.ndarray) -> np.ndarray:
    pts = np.ascontiguousarray(np.asarray(pts, dtype=np.float32))
    grid = np.ascontiguousarray(np.asarray(grid, dtype=np.float32))
    assert pts.shape == (N, 3) and grid.shape == (R3, D)
    nc = _get_nc()
    in_maps = [
        {"pts": pts[c * NSH:(c + 1) * NSH], "grid": grid}
        for c in range(NCORES)
    ]
    res = run_bass_kernel_spmd(nc, in_maps, list(range(NCORES)))
    out = np.concatenate([res.results[c]["out"] for c in range(NCORES)], axis=0)
    return out.astype(np.float32)



# revision 6
# speedup vs baseline: 2.0175x; 2.0175x over previous
"""BCC-lattice grid encoding (embedding lookup) on 8 Trainium2 NeuronCores.

Strategy: points batch-sharded across 8 cores, 512MB grid table replicated.
Per core: a fp32 DVE pipeline computes the 4 BCC tetrahedron vertex row
indices + barycentric weights bit-exactly vs the jax reference (verified:
indices match the reference exactly, final output max abs err ~9e-9 from
fp reassociation only), rows are gathered from HBM with indirect DMAs
(128 rows per instruction — the only int32-index gather primitive on this
HW; dma_gather needs int16 indices + 256B strides), and the weighted
combination runs on the vector engine.

Measured on trn2: ~10.4 ms/core (8 cores run the same program in parallel),
dominated by the 8192 indirect-DMA instructions/core at ~1.12-1.3 us each of
Q7 SWDGE descriptor-generation time; the gathered bytes themselves are only
~34 MB/core (~0.1 ms at HBM rate). Tuning notes from iteration: gather-buffer
depth dbufs=16 with cg=32 chunks beats shallower buffering (15.2 -> 10.4 ms);
issuing the 4 vertices' gathers v-major (all of d1's chunk, then d2's...)
beats k-major interleaving by 2x; keeping the gather stream phase-separated
from the index pipeline (whole-core idx tensors) beats per-tile interleaving
by 3x (fine-grained cross-engine waits throttle the Pool queue); a larger
SWDGE descriptor-ring carveout and multi-queue round-robin change nothing.
"""
import os
import sys

for _p in ("/opt/trn_rl_repo", "/root/.axon_site/_ro/trn_rl_repo"):
    if os.path.isdir(_p) and _p not in sys.path:
        sys.path.insert(0, _p)

import numpy as np
import concourse.bass as bass
import concourse.bacc as bacc
import concourse.mybir as mybir
import concourse.tile as tile
from concourse.bass_utils import run_bass_kernel_spmd

f32 = mybir.dt.float32
i32 = mybir.dt.int32
A = mybir.AluOpType

N = 2_097_152          # total points
NCORES = 8
NSH = N // NCORES      # points per core
P = 128                # SBUF partitions
R3 = 16_777_216        # grid rows (256^3)
D = 8                  # channels per row
MAGIC = 8388608.0      # 2^23: (x + MAGIC) - MAGIC == rne-round-to-int


def _build_nc(nsh=NSH, fc=256, cg=32, dbufs=16, scratch=16384):
    """Build the per-core SPMD program.

    nsh: points this core owns; fc: index-pipeline tile width (free dim per
    partition); cg: gather/interp chunk width.
    """
    T = nsh // P                      # free-dim points per partition
    assert T % fc == 0 and T % cg == 0

    nc = bacc.Bacc(dynamic_dma_scratch_size=scratch)
    pts_in = nc.declare_dram_parameter("pts", [nsh, 3], f32, isOutput=False)
    grid_in = nc.declare_dram_parameter("grid", [R3, D], f32, isOutput=False)
    out_dram = nc.declare_dram_parameter("out", [nsh, D], f32, isOutput=True)

    # DRAM views: partition p owns points [p*T, (p+1)*T)
    pts_v = pts_in[:].rearrange("(p t) c -> p (t c)", p=P)     # [128, T*3]
    out_v = out_dram[:].rearrange("(p t) c -> p (t c)", p=P)   # [128, T*8]

    with tile.TileContext(nc) as tc:
        with (
            tc.tile_pool(name="persist", bufs=1) as pp,
            tc.tile_pool(name="scratch", bufs=1) as sp,
            tc.tile_pool(name="io", bufs=2) as iop,
            tc.tile_pool(name="dp", bufs=dbufs) as dpool,
        ):

            def ts(out, in0, s1, op0, s2=None, op1=None):
                if s2 is None:
                    nc.vector.tensor_scalar(out=out, in0=in0, scalar1=s1,
                                            scalar2=None, op0=op0)
                else:
                    nc.vector.tensor_scalar(out=out, in0=in0, scalar1=s1,
                                            scalar2=s2, op0=op0, op1=op1)

            def tt(out, in0, in1, op):
                nc.vector.tensor_tensor(out=out, in0=in0, in1=in1, op=op)

            # whole-core persistent index/weight tensors (phase separation:
            # keeping the 8192-instruction gather stream free of fine-grained
            # cross-engine waits measures ~3x faster than interleaving)
            idx = [pp.tile([P, T], i32, name=f"idx{v}", tag=f"idx{v}")
                   for v in range(4)]
            wt = [pp.tile([P, T], f32, name=f"w{v}", tag=f"w{v}")
                  for v in range(4)]

            ntile = T // fc
            for j in range(ntile):
                sl = slice(j * fc, (j + 1) * fc)
                pts_t = iop.tile([P, fc * 3], f32, name="pts_t", tag="pts")
                nc.sync.dma_start(out=pts_t[:],
                                  in_=pts_v[:, j * fc * 3:(j + 1) * fc * 3])
                p3 = pts_t[:].rearrange("p (t c) -> p t c", c=3)

                def st(tag):
                    return sp.tile([P, fc], f32, name=tag, tag=tag)

                # stage A: coords -> abc -> floors/fracs -> t,u,w
                xs, ys, zs = st("xs"), st("ys"), st("zs")
                ts(xs[:], p3[:, :, 0], 255.5, A.mult)
                ts(ys[:], p3[:, :, 1], 255.5, A.mult)
                ts(zs[:], p3[:, :, 2], 127.5, A.mult)
                av, bv, cv = st("av"), st("bv"), st("cv")
                tt(av[:], xs[:], ys[:], A.add)
                tt(bv[:], xs[:], zs[:], A.add)
                tt(cv[:], ys[:], zs[:], A.add)

                # floor via magic rne + correction (values >= 0)
                def floor_to(dst, x, rr, gg):
                    ts(rr[:], x[:], MAGIC, A.add, MAGIC, A.subtract)
                    tt(gg[:], rr[:], x[:], A.is_gt)
                    tt(dst[:], rr[:], gg[:], A.subtract)

                fa, fb, fcr = st("fa"), st("fb"), st("fc")
                Fa, Fb, Fc = st("Fa"), st("Fb"), st("Fc")
                h1, h2 = st("h1"), st("h2")
                floor_to(Fa, av, h1, h2)
                tt(fa[:], av[:], Fa[:], A.subtract)
                floor_to(Fb, bv, h1, h2)
                tt(fb[:], bv[:], Fb[:], A.subtract)
                floor_to(Fc, cv, h1, h2)
                tt(fcr[:], cv[:], Fc[:], A.subtract)

                tv, uv, wv = st("tv"), st("uv"), st("wv")
                tt(h1[:], Fb[:], Fc[:], A.subtract)      # d = Fb-Fc
                tt(tv[:], Fa[:], h1[:], A.add)
                tt(uv[:], Fa[:], h1[:], A.subtract)
                tt(h2[:], Fb[:], Fc[:], A.add)           # s = Fb+Fc
                tt(wv[:], h2[:], Fa[:], A.subtract)

                # stage B: barycentric weights
                s1t, s3t, s2t = st("s1t"), st("s3t"), st("s2t")
                tt(s1t[:], fa[:], fb[:], A.max)
                tt(s1t[:], s1t[:], fcr[:], A.max)
                tt(s3t[:], fa[:], fb[:], A.min)
                tt(s3t[:], s3t[:], fcr[:], A.min)
                tt(s2t[:], fa[:], fb[:], A.add)
                tt(s2t[:], s2t[:], fcr[:], A.add)
                tt(s2t[:], s2t[:], s1t[:], A.subtract)
                tt(s2t[:], s2t[:], s3t[:], A.subtract)
                # out = w0*d1 + w1*d2 + w2*d3 + w3*d4
                ts(wt[0][:, sl], s1t[:], -1.0, A.mult, 1.0, A.add)   # 1-s1
                nc.vector.tensor_copy(out=wt[1][:, sl], in_=s3t[:])  # s3
                tt(wt[2][:, sl], s1t[:], s2t[:], A.subtract)         # s1-s2
                tt(wt[3][:, sl], s2t[:], s3t[:], A.subtract)         # s2-s3

                # stage C: argmax/argmin one-hots (first-index tie-break)
                e1a, e1b, qe = st("e1a"), st("e1b"), st("qe")
                tt(h1[:], fa[:], fb[:], A.is_ge)          # gab
                tt(h2[:], fa[:], fcr[:], A.is_ge)         # gac
                tt(e1a[:], h1[:], h2[:], A.mult)
                ts(h1[:], h1[:], -1.0, A.mult, 1.0, A.add)  # gba = 1-gab
                tt(h2[:], fb[:], fcr[:], A.is_ge)         # gbc
                tt(e1b[:], h1[:], h2[:], A.mult)
                tt(qe[:], e1a[:], e1b[:], A.add)          # e1c = 1-qe
                ma, mb, qm = st("ma"), st("mb"), st("qm")
                tt(h1[:], fa[:], fb[:], A.is_le)          # lab
                tt(h2[:], fa[:], fcr[:], A.is_le)         # lac
                tt(ma[:], h1[:], h2[:], A.mult)
                ts(h1[:], h1[:], -1.0, A.mult, 1.0, A.add)  # lba
                tt(h2[:], fb[:], fcr[:], A.is_le)         # lbc
                tt(mb[:], h1[:], h2[:], A.mult)
                tt(qm[:], ma[:], mb[:], A.add)            # mc = 1-qm

                # stage D helpers
                def cfh(dst, x, bias):
                    """dst = floor(clamp(x*0.5 + bias, 0, 255.5)) ; dst f32"""
                    if bias == 0.0:
                        ts(h1[:], x[:], 0.5, A.mult)
                    else:
                        ts(h1[:], x[:], 0.5, A.mult, bias, A.add)
                    ts(h1[:], h1[:], 255.5, A.min, 0.0, A.max)
                    ts(h2[:], h1[:], MAGIC, A.add, MAGIC, A.subtract)
                    tt(h3[:], h2[:], h1[:], A.is_gt)
                    tt(dst[:], h2[:], h3[:], A.subtract)

                h3, h4 = st("h3"), st("h4")
                i0c, i1c, i2c = st("i0c"), st("i1c"), st("i2c")

                def combine(v):
                    ts(h1[:], i0c[:], 65536.0, A.mult)
                    ts(h2[:], i1c[:], 256.0, A.mult)
                    tt(h1[:], h1[:], h2[:], A.add)
                    tt(h1[:], h1[:], i2c[:], A.add)
                    nc.vector.tensor_copy(out=idx[v][:, sl], in_=h1[:])

                # vertex 1: floors of (t/2, u/2), w
                cfh(i0c, tv, 0.0)
                cfh(i1c, uv, 0.0)
                ts(i2c[:], wv[:], 255.0, A.min, 0.0, A.max)
                combine(0)
                # vertex 2: +(1,1,1) -> ((t+1)/2, (u+1)/2, w+1)
                cfh(i0c, tv, 0.5)
                cfh(i1c, uv, 0.5)
                ts(i2c[:], wv[:], 1.0, A.add, 255.0, A.min)
                ts(i2c[:], i2c[:], 0.0, A.max)
                combine(1)
                # vertex 3: p1 + abc_to_xyz(e1):
                # dt=1-2*e1c=2*qe-1, du=1-2*e1b, dw=1-2*e1a
                ts(h4[:], qe[:], 2.0, A.mult, -1.0, A.add)
                tt(h4[:], tv[:], h4[:], A.add)
                cfh(i0c, h4, 0.0)
                ts(h4[:], e1b[:], -2.0, A.mult, 1.0, A.add)
                tt(h4[:], uv[:], h4[:], A.add)
                cfh(i1c, h4, 0.0)
                ts(h4[:], e1a[:], -2.0, A.mult, 1.0, A.add)
                tt(h4[:], wv[:], h4[:], A.add)
                ts(i2c[:], h4[:], 255.0, A.min, 0.0, A.max)
                combine(2)
                # vertex 4: +2*unit(argmin): dt=2*mc=2-2*qm, du=2*mb, dw=2*ma
                ts(h4[:], qm[:], -2.0, A.mult, 2.0, A.add)
                tt(h4[:], tv[:], h4[:], A.add)
                cfh(i0c, h4, 0.0)
                ts(h4[:], mb[:], 2.0, A.mult)
                tt(h4[:], uv[:], h4[:], A.add)
                cfh(i1c, h4, 0.0)
                ts(h4[:], ma[:], 2.0, A.mult)
                tt(h4[:], wv[:], h4[:], A.add)
                ts(i2c[:], h4[:], 255.0, A.min, 0.0, A.max)
                combine(3)

            # phase 2: gather + interpolate, chunk by chunk
            nchunk = T // cg
            for ci in range(nchunk):
                dts = [dpool.tile([P, cg * D], f32, name=f"d{v}",
                                  tag=f"d{v}") for v in range(4)]
                for v in range(4):
                    for k in range(cg):
                        col = ci * cg + k
                        nc.gpsimd.indirect_dma_start(
                            out=dts[v][:, k * D:(k + 1) * D],
                            out_offset=None,
                            in_=grid_in[:],
                            in_offset=bass.IndirectOffsetOnAxis(
                                ap=idx[v][:, col:col + 1], axis=0),
                        )
                oc = iop.tile([P, cg * D], f32, name="oc", tag="oc")
                t2 = iop.tile([P, cg * D], f32, name="t2i", tag="t2i")
                for v in range(4):
                    wb = wt[v][:, ci * cg:(ci + 1) * cg].unsqueeze(-1) \
                        .broadcast_to([P, cg, D])
                    dv3 = dts[v][:].rearrange("p (t c) -> p t c", c=D)
                    if v == 0:
                        tt(oc[:].rearrange("p (t c) -> p t c", c=D),
                           dv3, wb, A.mult)
                    else:
                        tt(t2[:].rearrange("p (t c) -> p t c", c=D),
                           dv3, wb, A.mult)
                        tt(oc[:], oc[:], t2[:], A.add)
                nc.sync.dma_start(out=out_v[:, ci * cg * D:(ci + 1) * cg * D],
                                  in_=oc[:])

    nc.compile()
    return nc


_NC_CACHE = {}


def _get_nc(key=(NSH, 256, 32, 16)):
    if key not in _NC_CACHE:
        _NC_CACHE[key] = _build_nc(*key)
    return _NC_CACHE[key]


def kernel(pts: np.ndarray, grid: np.ndarray) -> np.ndarray:
    pts = np.ascontiguousarray(np.asarray(pts, dtype=np.float32))
    grid = np.ascontiguousarray(np.asarray(grid, dtype=np.float32))
    assert pts.shape == (N, 3) and grid.shape == (R3, D)
    nc = _get_nc()
    in_maps = [
        {"pts": pts[c * NSH:(c + 1) * NSH], "grid": grid}
        for c in range(NCORES)
    ]
    res = run_bass_kernel_spmd(nc, in_maps, list(range(NCORES)))
    out = np.concatenate([res.results[c]["out"] for c in range(NCORES)], axis=0)
    return out.astype(np.float32)

